# revision 1
# baseline (speedup 1.0000x reference)
"""Causal self-attention on 8 TRN2 NeuronCores.

Sharding (per hint): batch x head-group mesh (4 x 2). Core c handles
batch b = c//2 and head group g = c%2 (8 of 16 heads). Each core:
  qkv = x_b @ W_attn[:, cols(g)]          (fp32r matmuls, full rate)
  per head: S^T = k^T.T @ q^T, P = exp(S/8) causal, y = P@v (ones-row
  trick gives the softmax denominator for free), y /= l
  partial_out = y @ W_proj[rows(g)]
  ReduceScatter over pairs {2b, 2b+1} sums the two head-group partials;
  core 2b keeps t[0:1024], core 2b+1 keeps t[1024:2048].

kernel(**inputs) takes FULL inputs, shards on host, runs the SPMD bass
kernel on cores 0-7, reassembles the full [4, 2048, 1024] output.
"""

from contextlib import ExitStack

import numpy as np

import concourse.bass as bass
import concourse.tile as tile
from concourse import bacc, mybir
from concourse.bass_utils import run_bass_kernel_spmd

F32 = mybir.dt.float32
F32R = mybir.dt.float32r
BF16 = mybir.dt.bfloat16
AF = mybir.ActivationFunctionType

D = 1024          # model dim
T = 2048          # sequence length
B = 4             # batch
HD = 64           # head dim
NH = 8            # heads per core
DC = D // 128     # 8 contraction chunks
TT = T // 128     # 16 t-tiles
SCALE = 1.0 / 8.0  # 1/sqrt(HD)


def _pieces(width):
    """Split width into matmul pieces <=512, PSUM-bank aligned from offset 0."""
    out = []
    off = 0
    while off < width:
        n = min(512, width - off)
        out.append((off, n))
        off += n
    return out


def build(repeat=1, collective=True):
    nc = bacc.Bacc("TRN2", target_bir_lowering=False, debug=False, num_devices=8)

    x_ext = nc.dram_tensor("x", [T, D], F32, kind="ExternalInput").ap()
    wa_ext = nc.dram_tensor("wa", [D, 3 * 512], F32, kind="ExternalInput").ap()
    wp_ext = nc.dram_tensor("wp", [512, D], F32, kind="ExternalInput").ap()
    ident_ext = nc.dram_tensor("ident", [128, 128], F32, kind="ExternalInput").ap()
    mask_ext = nc.dram_tensor("trimask", [128, 128], F32, kind="ExternalInput").ap()
    out_ext = nc.dram_tensor("out_shard", [T // 2, D], F32, kind="ExternalOutput").ap()

    with tile.TileContext(nc) as tc, ExitStack() as top:
        # ---- persistent pools ----
        pers = top.enter_context(tc.tile_pool(name="pers", bufs=1))
        dram = top.enter_context(tc.tile_pool(name="dram", bufs=1, space="DRAM"))

        qkT = [pers.tile([128, T], BF16, tag=f"qkT{i}", name=f"qkT{i}") for i in range(8)]
        # v_sb[tt]: [128 k-parts, 8 heads, 64 v + 1 ones] bf16
        v_sb = [pers.tile([128, NH, HD + 1], BF16, tag=f"v{i}", name=f"v{i}") for i in range(TT)]
        ident_sb = pers.tile([128, 128], F32, tag="ident")
        mask_bf = pers.tile([128, 128], BF16, tag="maskbf")
        prj_bounce = dram.tile([T, D], F32)
        rs_bounce = dram.tile([T // 2, D], F32)
        # per-(head, q-half) DRAM rows used to broadcast 1/l across partitions
        l_dram = dram.tile([16, 1024], F32)

        mask_f32 = pers.tile([128, 128], F32, tag="maskf32")
        nc.sync.dma_start(ident_sb[:], ident_ext)
        nc.sync.dma_start(mask_f32[:], mask_ext)
        nc.vector.tensor_copy(mask_bf[:], mask_f32[:])

        def body(iv=None):
            # ================= phase 1: transpose x, QKV =================
            with ExitStack() as ph1:
                p1 = ph1.enter_context(tc.tile_pool(name="p1", bufs=1))
                xstage = ph1.enter_context(tc.tile_pool(name="xstage", bufs=2))
                wstage = ph1.enter_context(tc.tile_pool(name="wstage", bufs=2))
                trps = ph1.enter_context(
                    tc.tile_pool(name="trps", bufs=2, space="PSUM"))
                qkvps = ph1.enter_context(
                    tc.tile_pool(name="qkvps", bufs=3, space="PSUM"))

                xT = [p1.tile([128, T], F32R, tag=f"xT{i}", name=f"xT{i}") for i in range(DC)]

                # --- transpose x into xT (PE transpose, 2 t-tiles per copy) ---
                for tp in range(TT // 2):  # pairs of t-tiles
                    xt = [xstage.tile([128, D], F32, tag="xt", name="xt") for _ in range(2)]
                    for j in range(2):
                        t0 = (2 * tp + j) * 128
                        nc.sync.dma_start(xt[j][:], x_ext[t0:t0 + 128, :])
                    for dc in range(DC):
                        tr = trps.tile([128, 256], F32, tag="tr")
                        for j in range(2):
                            nc.tensor.transpose(
                                tr[:, j * 128:(j + 1) * 128],
                                xt[j][:, dc * 128:(dc + 1) * 128],
                                ident_sb[:])
                        nc.any.tensor_copy(
                            xT[dc][:, tp * 256:(tp + 1) * 256], tr[:])

                # --- wa_v: v columns of wa, converted to f32r once ---
                wa_v = [p1.tile([128, 512], F32R, tag=f"wav{dc}", name=f"wav{dc}")
                        for dc in range(DC)]
                for dc in range(DC):
                    wv = xstage.tile([128, 512], F32, tag="wvstage")
                    nc.sync.dma_start(
                        wv[:], wa_ext[dc * 128:(dc + 1) * 128, 1024:1536])
                    nc.any.tensor_copy(wa_v[dc][:], wv[:])

                # --- q/k col-tiles: qkT[ct] = wa[:, ct].T @ x.T ---
                # emit in order q0,k0,q1,k1,... so early heads finish first
                for cti, ct in enumerate([0, 4, 1, 5, 2, 6, 3, 7]):
                    wqk = wstage.tile([128, 8, 128], F32, tag="wqk")
                    # DRAM AP: [p(128, stride 3*512), dc(8, stride 128*3*512),
                    #           j(128, stride 1)]
                    src = wa_ext[:, ct * 128:(ct + 1) * 128].rearrange(
                        "(c p) n -> p c n", p=128)
                    nc.sync.dma_start(wqk[:], src)
                    wqk_r = wstage.tile([128, 8, 128], F32R, tag="wqkr")
                    nc.any.tensor_copy(wqk_r[:], wqk[:])
                    for tch in range(4):
                        ps = qkvps.tile([128, 512], F32, tag="qkvps")
                        for dc in range(DC):
                            nc.tensor.matmul(
                                ps[:],
                                wqk_r[:, dc, :],
                                xT[dc][:, tch * 512:(tch + 1) * 512],
                                start=(dc == 0), stop=(dc == DC - 1))
                        nc.any.tensor_copy(
                            qkT[ct][:, tch * 512:(tch + 1) * 512], ps[:])

                # --- v natural: v[t-tile] = x[t-tile] @ wa_v ---
                for tt in range(TT):
                    ps = qkvps.tile([128, 512], F32, tag="qkvps")
                    for dc in range(DC):
                        nc.tensor.matmul(
                            ps[:],
                            xT[dc][:, tt * 128:(tt + 1) * 128],
                            wa_v[dc][:],
                            start=(dc == 0), stop=(dc == DC - 1))
                    nc.any.tensor_copy(
                        v_sb[tt][:, :, 0:HD],
                        ps[:].rearrange("p (h d) -> p h d", h=NH))
                    nc.vector.memset(v_sb[tt][:, :, HD:HD + 1], 1.0)

            # ================= phase 2: attention =================
            with ExitStack() as ph23:
                p23 = ph23.enter_context(tc.tile_pool(name="p23", bufs=1))
                yT = [p23.tile([128, T], F32R, tag=f"yT{i}", name=f"yT{i}")
                      for i in range(4)]
                ph2 = ph23.enter_context(ExitStack())
                sps = ph2.enter_context(
                    tc.tile_pool(name="sps", bufs=2, space="PSUM"))
                yps = ph2.enter_context(
                    tc.tile_pool(name="yps", bufs=2, space="PSUM"))
                ppool = ph2.enter_context(tc.tile_pool(name="ppool", bufs=3))
                npool = ph2.enter_context(tc.tile_pool(name="npool", bufs=2))

                for h in range(NH):
                    ct_q, ct_k = h // 2, 4 + h // 2
                    p0 = 64 * (h % 2)
                    qT_h = qkT[ct_q][p0:p0 + 64, :]
                    kT_h = qkT[ct_k][p0:p0 + 64, :]
                    for qh in range(2):
                        qbase = 1024 * qh
                        y_ps = yps.tile([65, 1024], F32, tag="y")
                        n_k = 8 * qh + 8
                        for i in range(n_k):
                            q0 = max(qbase, 128 * i)
                            w = qbase + 1024 - q0
                            s_ps = sps.tile([128, w], F32, tag="s")
                            for off, n in _pieces(w):
                                nc.tensor.matmul(
                                    s_ps[:, off:off + n],
                                    kT_h[:, 128 * i:128 * (i + 1)],
                                    qT_h[:, q0 + off:q0 + off + n],
                                    start=True, stop=True)
                            p_sb = ppool.tile([128, 1024], BF16, tag="p")
                            nc.scalar.activation(
                                p_sb[:, 0:w], s_ps[:], AF.Exp, scale=SCALE)
                            if 128 * i >= qbase:  # diagonal block: causal mask
                                nc.vector.tensor_mul(
                                    p_sb[:, 0:128], p_sb[:, 0:128], mask_bf[:])
                            # AV pieces: align to y_ps's 512-wide PSUM banks
                            yoff = q0 - qbase
                            aoff = 0
                            while aoff < w:
                                n = min(512 - (yoff + aoff) % 512, w - aoff)
                                bank = (yoff + aoff) // 512
                                nc.tensor.matmul(
                                    y_ps[:, yoff + aoff:yoff + aoff + n],
                                    v_sb[i][:, h, :],
                                    p_sb[:, aoff:aoff + n],
                                    start=(i == 0),
                                    stop=(i == 8 * qh + 4 * bank + 3))
                                aoff += n
                        # normalize: yT[h//2][64*(h%2):, qbase:+1024] = y/l
                        # recip stays on lane 64 (DVE is lane-locked), then
                        # gpsimd broadcasts it to lanes 0-63.
                        l_sb = npool.tile([65, 1024], F32, tag="lsb")
                        nc.vector.reciprocal(l_sb[64:65, :], y_ps[64:65, :])
                        recip_b = npool.tile([64, 1024], F32, tag="recipb")
                        slot = 2 * h + qh
                        nc.sync.dma_start(
                            l_dram[slot:slot + 1, :], l_sb[64:65, :])
                        nc.sync.dma_start(
                            recip_b[:],
                            l_dram[slot:slot + 1, :].partition_broadcast(64))
                        if h % 2 == 0:
                            nc.vector.tensor_mul(
                                yT[h // 2][0:64, qbase:qbase + 1024],
                                y_ps[0:64, :], recip_b[:])
                        else:
                            ytmp = npool.tile([64, 1024], F32R, tag="ytmp")
                            nc.vector.tensor_mul(
                                ytmp[:], y_ps[0:64, :], recip_b[:])
                            nc.sync.dma_start(
                                yT[h // 2][64:128, qbase:qbase + 1024],
                                ytmp[:])

                ph2.close()

                # ================= phase 3: proj =================
                ph3 = ph23.enter_context(ExitStack())
                p3 = ph3.enter_context(tc.tile_pool(name="p3", bufs=1))
                wstage3 = ph3.enter_context(tc.tile_pool(name="wstage3", bufs=2))
                ops = ph3.enter_context(
                    tc.tile_pool(name="ops", bufs=6, space="PSUM"))
                opool = ph3.enter_context(tc.tile_pool(name="opool", bufs=3))

                wp_r = [p3.tile([128, D], F32R, tag=f"wp{dc}", name=f"wpr{dc}") for dc in range(4)]
                for dc in range(4):
                    ws = wstage3.tile([128, D], F32, tag="wps")
                    nc.sync.dma_start(ws[:], wp_ext[dc * 128:(dc + 1) * 128, :])
                    nc.any.tensor_copy(wp_r[dc][:], ws[:])

                for tt in range(TT):
                    o_ps = ops.tile([128, 512], F32, tag="o")
                    o_ps2 = ops.tile([128, 512], F32, tag="o")
                    for oc, ps in enumerate([o_ps, o_ps2]):
                        for dc in range(4):
                            nc.tensor.matmul(
                                ps[:],
                                yT[dc][:, tt * 128:(tt + 1) * 128],
                                wp_r[dc][:, oc * 512:(oc + 1) * 512],
                                start=(dc == 0), stop=(dc == 3))
                    o_sb = opool.tile([128, D], F32, tag="osb")
                    nc.any.tensor_copy(o_sb[:, 0:512], o_ps[:])
                    nc.any.tensor_copy(o_sb[:, 512:1024], o_ps2[:])
                    nc.sync.dma_start(
                        prj_bounce[tt * 128:(tt + 1) * 128, :], o_sb[:])

        if repeat == 1:
            body()
        else:
            with tc.For_i(0, repeat, 1) as iv:
                body(iv)

        # ================= reduce-scatter + output =================
        if not collective:
            nc.sync.dma_start(out_ext, prj_bounce[0:T // 2, :])
        if collective:
            nc.gpsimd.collective_compute(
                "ReduceScatter",
                mybir.AluOpType.add,
                replica_groups=[[0, 1], [2, 3], [4, 5], [6, 7]],
                ins=[prj_bounce.opt()],
                outs=[rs_bounce.opt()],
            )
            nc.sync.dma_start(out_ext, rs_bounce[:])

    nc.compile()
    return nc


def make_in_maps(x, W_attn, W_proj):
    ident = np.eye(128, dtype=np.float32)
    trimask = np.triu(np.ones((128, 128), dtype=np.float32))
    in_maps = []
    for c in range(8):
        b, g = c // 2, c % 2
        wa = np.concatenate(
            [W_attn[:, 512 * g:512 * g + 512],
             W_attn[:, 1024 + 512 * g:1024 + 512 * g + 512],
             W_attn[:, 2048 + 512 * g:2048 + 512 * g + 512]], axis=1)
        in_maps.append({
            "x": np.ascontiguousarray(x[b]),
            "wa": np.ascontiguousarray(wa),
            "wp": np.ascontiguousarray(W_proj[512 * g:512 * g + 512, :]),
            "ident": ident,
            "trimask": trimask,
        })
    return in_maps


_NC_CACHE = {}


def kernel(x, W_attn, W_proj):
    x = np.asarray(x, dtype=np.float32)
    W_attn = np.asarray(W_attn, dtype=np.float32)
    W_proj = np.asarray(W_proj, dtype=np.float32)
    if "nc" not in _NC_CACHE:
        _NC_CACHE["nc"] = build()
    nc = _NC_CACHE["nc"]
    in_maps = make_in_maps(x, W_attn, W_proj)
    res = run_bass_kernel_spmd(nc, in_maps, list(range(8)))
    out = np.empty((B, T, D), dtype=np.float32)
    for c in range(8):
        b, g = c // 2, c % 2
        out[b, 1024 * g:1024 * (g + 1), :] = res.results[c]["out_shard"]
    return out

